# revision 43
# baseline (speedup 1.0000x reference)
"""Trainium2 Bass kernel for the CustomODELoss problem.

Full inputs:
    predicted_solution_batch [4096, 8192] f32
    target_solution_batch    [4096, 8192] f32
    c_input_batch            [4096]       f32
    x_eval_points            [8192]       f32   (uniform grid on [0, 1])

loss = mean((pred - target)^2)
     + mean((pred[r, idx_r] - 1)^2)
     + mean(((pred[r, idx_p] - pred[r, idx_m]) / ((idx_p - idx_m) * dx))^2)
where idx_r = argmin_j |x_j - c_r| (first index on ties).

Sharding: data-parallel over the batch dim, 512 rows per core on 8 cores.
Each core streams its pred/targ slice once (memory-bound integral term)
and resolves the per-row grid index + finite-difference gather on device
via indirect DMA.

Key structure (from trace analysis):
  * The HWDGE sync ring sustains ~420 GB/s (16 DMA engines x ~26 GB/s)
    regardless of packet size >= 2KB.  1MB (128x2048) descriptors are the
    sweet spot: 4MB descriptors drop the ring to ~320 GB/s and fire their
    completion semaphores ~2 descriptors late (round-robin expansion);
    small lead-in descriptors engage fewer engines and slow the ramp.
  * Streaming compute: DVE subtract in place, ACT Square with accum_out
    per tile.  The last row block tapers 2048->128 so the post-stream
    serial tail (last load -> sub -> square -> reduce -> store) is short;
    the final tile's square+sum runs on DVE while ACT drains the
    second-to-last accumulates.
  * The index resolve is exact: candidate j0 = int(c*(N-1)) is within 1
    of the true argmin; the three candidate grid values are computed
    ARITHMETICALLY as (jc+e)*dx, which reproduces linspace(0,1,N) f32
    bit-exactly for every j (verified on the full grid), so no x gather
    is needed.  Distance comparison via squares preserves order/ties;
    first-index tie-break matches jnp.argmin.  A 5-wide pred window
    gathered by indirect DMA covers every (idx-1, idx, idx+1) triple.
  * dx is baked into the program as an immediate (a [128,1] DMA for it
    completed ~6us late behind the stream's 8KB packets and stalled the
    index chain); c loads via the sync HWDGE ring ahead of the stream
    (SWDGE would land it ~5us later); pred-window gathers on gpsimd
    SWDGE.  The pw-dependent select chain is anchored mid-stream via a
    bit-exact +0 so a late gather can never stall the in-order DVE queue
    between early stream subtracts.
  * The device emits per-partition partial sums; the host sums the
    8x128 partials and forms the three means.
"""

import numpy as np

import concourse.bacc as bacc
import concourse.bass as bass
import concourse.mybir as mybir
from concourse import tile
from concourse.bass_utils import run_bass_kernel_spmd

F32 = mybir.dt.float32
I32 = mybir.dt.int32
OP = mybir.AluOpType

B = 4096
N = 8192
NCORES = 8
BL = B // NCORES          # rows per core = 512
P = 128                   # SBUF partitions
RB = BL // P              # row groups per partition = 4
FT = 2048                 # max free-dim tile width for the streaming phase
W = 5                     # pred-window width
PBUF = 12                 # stream pool depth (pred/targ each)

# Streaming tile schedule: (row_block, col_start, width) per pred/targ pair.
# The last row block tapers to tiny tiles so the serial pipeline tail
# (last load -> sub -> square -> reduce -> store) is short; everything
# before the taper stays full-width so the stream keeps large transfers.
# (Tried and LOSES: 4MB row-block descriptors — the ring's round-robin
# expansion fires completion semaphores ~2 descriptors late and sustained
# bandwidth drops ~420 -> ~320 GB/s; small ramp-in tiles — the ring
# expands only ~4 descriptors concurrently, so small first descriptors
# engage FEWER engines and slow the ramp.)
_taper = [2048, 2048, 2048, 1024, 512, 256, 128, 64, 64]
assert sum(_taper) == N
TILES = [(_rb, _c * FT, FT) for _rb in range(BL // P - 1) for _c in range(N // FT)]
_cs = 0
for _w in _taper:
    TILES.append((BL // P - 1, _cs, _w))
    _cs += _w
NT = len(TILES)  # 20


def build_nc(dx, debug=False):
    # Bacc (not plain Bass): its compile pipeline runs
    # generate_event_semaphores, which splits multi-sem waits into separate
    # event instructions — TRN2 allows at most 1 embedded wait per
    # instruction, and walrus codegen rejects the unsplit form.
    nc = bacc.Bacc()

    pred = nc.dram_tensor("pred", [BL, N], F32, kind="ExternalInput")
    targ = nc.dram_tensor("targ", [BL, N], F32, kind="ExternalInput")
    # c per core, reshaped host-side to [128, 4]: row r = p*RB + q
    cvec = nc.dram_tensor("cvec", [P, RB], F32, kind="ExternalInput")
    # raw per-slice partials: cols 0..NT-1 = integral-term slice sums,
    # col NT = sum (f(c)-1)^2, col NT+1 = sum f'(c)^2.  The host sums
    # 8x128 partials anyway, so no on-device final reduce is needed —
    # that keeps the post-stream critical path one DVE/ACT op shorter.
    partials = nc.dram_tensor("partials", [P, NT + 2], F32, kind="ExternalOutput")
    if debug:
        dbg = nc.dram_tensor("dbg", [P, 56], F32, kind="ExternalOutput")

    def view3(t):  # [128, 12] tile -> [128, 4, 3] AP
        return t[:].rearrange("p (q k) -> p q k", k=3)

    def view5(t):  # [128, 20] tile -> [128, 4, 5] AP
        return t[:].rearrange("p (q k) -> p q k", k=5)

    with tile.TileContext(nc) as tc:
        with (
            tc.tile_pool(name="ppool", bufs=PBUF) as ppool,
            tc.tile_pool(name="tpool", bufs=PBUF) as tpool,
            tc.tile_pool(name="pb", bufs=1) as pb,
        ):
            # ========== phase A: stream sum((p-t)^2) ====================
            # All stream loads ride the sync HWDGE ring, which carries only
            # pure DMA dispatches and never starves.  (Putting targ loads on
            # the scalar ring was tried and LOSES: the two queues share the
            # same 16 DMA engines, and queue arbitration dips the combined
            # throughput while both are active.)
            parts1 = pb.tile([P, NT + 2], F32)

            tts = []

            def stream_pair(k):
                rb, cs, w = TILES[k]
                rs = rb * P
                pt = ppool.tile([P, FT], F32)
                tt = tpool.tile([P, FT], F32)
                tts.append(tt)
                nc.sync.dma_start(pt[:, :w], pred[rs:rs + P, cs:cs + w])
                if k == 0:
                    # tile 0's targ load warms the ACT HWDGE ring (neutral
                    # for the stream — verified) so the final store below
                    # doesn't pay the ring's cold-start latency.
                    nc.scalar.dma_start(tt[:, :w], targ[rs:rs + P, cs:cs + w])
                else:
                    nc.sync.dma_start(tt[:, :w], targ[rs:rs + P, cs:cs + w])
                # in-place pt <- pt - tt (the freed third pool doubles the
                # stream-pool depth instead)
                nc.vector.tensor_tensor(out=pt[:, :w], in0=pt[:, :w],
                                        in1=tt[:, :w], op=OP.subtract)
                if k == NT - 1:
                    # Last tile's square-sum on DVE: the serial tail after
                    # the final DMA byte is then DVE(sub+mult+reduce) in
                    # parallel with ACT draining the second-to-last tile,
                    # instead of ACT serializing two more sq+accum-read
                    # pairs behind it.
                    nc.vector.tensor_tensor(out=pt[:, :w], in0=pt[:, :w],
                                            in1=pt[:, :w], op=OP.mult)
                    nc.vector.reduce_sum(out=parts1[:, k:k + 1],
                                         in_=pt[:, :w],
                                         axis=mybir.AxisListType.X)
                else:
                    # pt <- pt^2 in place; accum_out = row-sum
                    nc.scalar.activation(
                        out=pt[:, :w], in_=pt[:, :w],
                        func=mybir.ActivationFunctionType.Square,
                        accum_out=parts1[:, k:k + 1],
                    )

            stream_pair(0)

            # ========== phase B part 1: indices + pred-window gather =====
            # c rides the sync HWDGE ring right AFTER pred0's dispatch:
            # via SWDGE it lands only ~13.5us in, which holds up the index
            # chain -> pw gathers -> the pw-gated part-2 DVE ops — via the
            # sync ring c lands ~9.5us and the resolve finishes with slack.
            # pred0 dispatches first so the stream is not delayed.
            c_t = pb.tile([P, RB], F32)
            nc.sync.dma_start(out=c_t[:], in_=cvec[:, :])

            # j0 = int(c * (N-1)); any convert rounding mode keeps
            # |j0 - argmin| <= 1, which the 3-candidate check fixes.
            u = pb.tile([P, RB], F32)
            nc.vector.tensor_scalar(out=u[:], in0=c_t[:], scalar1=float(N - 1),
                                    scalar2=None, op0=OP.mult)
            j0i = pb.tile([P, RB], I32)
            nc.vector.tensor_copy(out=j0i[:], in_=u[:])
            j0f = pb.tile([P, RB], F32)
            nc.vector.tensor_copy(out=j0f[:], in_=j0i[:])
            jcc = pb.tile([P, RB], F32)
            nc.vector.tensor_scalar(out=jcc[:], in0=j0f[:], scalar1=1.0,
                                    scalar2=float(N - 2), op0=OP.max, op1=OP.min)

            # pred window start: clip(j0-2, 0, N-W) — the 5-wide window
            # covers {jm, jstar, jp} for every jstar in {j0-1, j0, j0+1}.
            s5f = pb.tile([P, RB], F32)
            nc.vector.tensor_scalar(out=s5f[:], in0=j0f[:], scalar1=-2.0,
                                    scalar2=0.0, op0=OP.add, op1=OP.max)
            s5c = pb.tile([P, RB], F32)
            nc.vector.tensor_scalar(out=s5c[:], in0=s5f[:],
                                    scalar1=float(N - W), scalar2=None,
                                    op0=OP.min)
            s5i = pb.tile([P, RB], I32)
            nc.vector.tensor_copy(out=s5i[:], in_=s5c[:])
            rowbase = pb.tile([P, RB], I32)  # (p*RB + q) * N
            nc.gpsimd.iota(rowbase[:], pattern=[[N, RB]], base=0,
                           channel_multiplier=RB * N)
            offs = pb.tile([P, RB], I32)
            nc.vector.tensor_tensor(out=offs[:], in0=rowbase[:], in1=s5i[:],
                                    op=OP.add)

            # NOTE: hardware SWDGE honors only ONE offset per partition in an
            # indirect DMA (CoreSim accepts [128, RB] offsets, HW does not) —
            # issue one gather per row-group with [128, 1] offsets.
            pw = pb.tile([P, RB * W], F32)
            for q in range(RB):
                nc.gpsimd.indirect_dma_start(
                    out=pw[:, W * q:W * q + W], out_offset=None,
                    in_=pred[:, :],
                    in_offset=bass.IndirectOffsetOnAxis(
                        ap=offs[:, q:q + 1], axis=1),
                )

            iota15 = pb.tile([P, RB * W], F32)
            nc.gpsimd.iota(iota15[:], pattern=[[0, RB], [1, W]], base=0,
                           channel_multiplier=0,
                           allow_small_or_imprecise_dtypes=True)

            SPLIT = 12
            for k in range(1, SPLIT):
                stream_pair(k)

            # Anchor for the pw-dependent ops below: a zero tile whose input
            # is stream pair 8's targ buffer (read-only — no WAR on the
            # in-place subtract).  pw2 = pw + 0 is bit-identical to pw, but
            # it forces the scheduler to place the one-hot select chain
            # mid-stream (~45us), when the SWDGE gathers are guaranteed
            # complete even on a bad day.  Without it the in-order DVE queue
            # holds pw-gated ops right after the first stream subtract, and
            # a late gather stalls DVE -> the op-count dispatch guards fire
            # late -> the ring starves -> completion lateness compounds
            # (the bimodal ~97 vs ~114 us runs).
            z20 = pb.tile([P, RB * W], F32)
            nc.vector.tensor_scalar(out=z20[:], in0=tts[8][:, :RB * W],
                                    scalar1=0.0, scalar2=None, op0=OP.mult)
            pw2 = pb.tile([P, RB * W], F32)
            nc.vector.tensor_tensor(out=pw2[:], in0=pw[:], in1=z20[:],
                                    op=OP.add)

            # ========== phase B part 2: select + finite difference ======
            # Candidate grid values computed arithmetically: x_j = fl(j*dx)
            # bit-exactly reproduces linspace(0,1,N) f32 for every j
            # (verified on the full grid), so no x gather is needed — the
            # three candidates {jc-1, jc, jc+1} are (jcc+e)*dx.
            xw = pb.tile([P, RB * 3], F32)
            for e in range(3):
                nc.vector.tensor_scalar(out=xw[:, e::3], in0=jcc[:],
                                        scalar1=float(e - 1),
                                        scalar2=dx,
                                        op0=OP.add, op1=OP.mult)
            # Pure DVE (no ACT hop): |d| comparisons use d*d — f32 squaring
            # is monotone in |d|, so order and ties match abs comparison.
            dsb = pb.tile([P, RB * 3], F32)
            nc.vector.tensor_tensor(out=view3(dsb), in0=view3(xw),
                                    in1=c_t[:].to_broadcast([P, RB, 3]),
                                    op=OP.subtract)
            dsq = pb.tile([P, RB * 3], F32)
            nc.vector.tensor_tensor(out=dsq[:], in0=dsb[:], in1=dsb[:],
                                    op=OP.mult)
            dm, d0, dp = dsq[:, 0::3], dsq[:, 1::3], dsq[:, 2::3]

            # first-argmin among {jc-1, jc, jc+1}:
            #   a = (dm<=d0)&(dm<=dp); b = (1-a)&(d0<=dp)
            #   jstar = jc + 1 - 2a - b
            t1b = pb.tile([P, RB], F32)
            nc.vector.tensor_tensor(out=t1b[:], in0=dm, in1=d0, op=OP.is_le)
            t2b = pb.tile([P, RB], F32)
            nc.vector.tensor_tensor(out=t2b[:], in0=dm, in1=dp, op=OP.is_le)
            a_t = pb.tile([P, RB], F32)
            nc.vector.tensor_tensor(out=a_t[:], in0=t1b[:], in1=t2b[:],
                                    op=OP.mult)
            t3b = pb.tile([P, RB], F32)
            nc.vector.tensor_tensor(out=t3b[:], in0=d0, in1=dp, op=OP.is_le)
            oma = pb.tile([P, RB], F32)
            nc.vector.tensor_scalar(out=oma[:], in0=a_t[:], scalar1=-1.0,
                                    scalar2=1.0, op0=OP.mult, op1=OP.add)
            b_t = pb.tile([P, RB], F32)
            nc.vector.tensor_tensor(out=b_t[:], in0=t3b[:], in1=oma[:],
                                    op=OP.mult)
            e1 = pb.tile([P, RB], F32)
            nc.vector.tensor_scalar(out=e1[:], in0=a_t[:], scalar1=-2.0,
                                    scalar2=1.0, op0=OP.mult, op1=OP.add)
            e2 = pb.tile([P, RB], F32)
            nc.vector.tensor_tensor(out=e2[:], in0=e1[:], in1=b_t[:],
                                    op=OP.subtract)
            jstar = pb.tile([P, RB], F32)
            nc.vector.tensor_tensor(out=jstar[:], in0=jcc[:], in1=e2[:],
                                    op=OP.add)

            # neighbors and in-window positions relative to s5
            jm = pb.tile([P, RB], F32)
            nc.vector.tensor_scalar(out=jm[:], in0=jstar[:], scalar1=-1.0,
                                    scalar2=0.0, op0=OP.add, op1=OP.max)
            jp = pb.tile([P, RB], F32)
            nc.vector.tensor_scalar(out=jp[:], in0=jstar[:], scalar1=1.0,
                                    scalar2=float(N - 1), op0=OP.add, op1=OP.min)
            p0 = pb.tile([P, RB], F32)
            nc.vector.tensor_tensor(out=p0[:], in0=jstar[:], in1=s5c[:],
                                    op=OP.subtract)
            pmp = pb.tile([P, RB], F32)
            nc.vector.tensor_tensor(out=pmp[:], in0=jm[:], in1=s5c[:],
                                    op=OP.subtract)
            ppp = pb.tile([P, RB], F32)
            nc.vector.tensor_tensor(out=ppp[:], in0=jp[:], in1=s5c[:],
                                    op=OP.subtract)

            # f(c): one-hot select of window position jstar
            m0 = pb.tile([P, RB * W], F32)
            nc.vector.tensor_tensor(out=view5(m0), in0=view5(iota15),
                                    in1=p0[:].to_broadcast([P, RB, W]),
                                    op=OP.is_equal)
            pr0 = pb.tile([P, RB * W], F32)
            nc.vector.tensor_tensor(out=pr0[:], in0=m0[:], in1=pw2[:],
                                    op=OP.mult)
            fpc = pb.tile([P, RB], F32)
            nc.vector.reduce_sum(out=fpc[:], in_=view5(pr0),
                                 axis=mybir.AxisListType.X)

            # f'(c): (pred[jp] - pred[jm]) / ((jp-jm)*dx) via +/- one-hot
            mp_ = pb.tile([P, RB * W], F32)
            nc.vector.tensor_tensor(out=view5(mp_), in0=view5(iota15),
                                    in1=ppp[:].to_broadcast([P, RB, W]),
                                    op=OP.is_equal)
            mm_ = pb.tile([P, RB * W], F32)
            nc.vector.tensor_tensor(out=view5(mm_), in0=view5(iota15),
                                    in1=pmp[:].to_broadcast([P, RB, W]),
                                    op=OP.is_equal)
            wd = pb.tile([P, RB * W], F32)
            nc.vector.tensor_tensor(out=wd[:], in0=mp_[:], in1=mm_[:],
                                    op=OP.subtract)
            prd = pb.tile([P, RB * W], F32)
            nc.vector.tensor_tensor(out=prd[:], in0=wd[:], in1=pw2[:],
                                    op=OP.mult)
            df = pb.tile([P, RB], F32)
            nc.vector.reduce_sum(out=df[:], in_=view5(prd),
                                 axis=mybir.AxisListType.X)
            qd = pb.tile([P, RB], F32)
            nc.vector.tensor_tensor(out=qd[:], in0=jp[:], in1=jm[:],
                                    op=OP.subtract)
            den = pb.tile([P, RB], F32)
            nc.vector.tensor_scalar(out=den[:], in0=qd[:], scalar1=dx,
                                    scalar2=None, op0=OP.mult)
            rden = pb.tile([P, RB], F32)
            nc.vector.reciprocal(out=rden[:], in_=den[:])
            fpp = pb.tile([P, RB], F32)
            nc.vector.tensor_tensor(out=fpp[:], in0=df[:], in1=rden[:],
                                    op=OP.mult)

            # per-partition sums of (f(c)-1)^2 and f'(c)^2, straight into
            # the coalesced output tile.  (tensor_tensor_reduce compiles
            # but dies at runtime on HW — use ACT Square with accumulate
            # instead; these are terminal outputs, so the ACT-stream
            # position doesn't gate anything.)
            fpm1 = pb.tile([P, RB], F32)
            nc.vector.tensor_scalar(out=fpm1[:], in0=fpc[:], scalar1=-1.0,
                                    scalar2=None, op0=OP.add)
            sq2 = pb.tile([P, RB], F32)
            nc.scalar.activation(out=sq2[:], in_=fpm1[:],
                                 func=mybir.ActivationFunctionType.Square,
                                 accum_out=parts1[:, NT:NT + 1])
            sq3 = pb.tile([P, RB], F32)
            nc.scalar.activation(out=sq3[:], in_=fpp[:],
                                 func=mybir.ActivationFunctionType.Square,
                                 accum_out=parts1[:, NT + 1:NT + 2])

            if debug:
                dbt = pb.tile([P, 56], F32)
                nc.vector.tensor_copy(out=dbt[:, 0:12], in_=xw[:])
                nc.vector.tensor_copy(out=dbt[:, 12:32], in_=pw[:])
                nc.vector.tensor_copy(out=dbt[:, 32:36], in_=jstar[:])
                nc.vector.tensor_copy(out=dbt[:, 36:40], in_=s5c[:])
                nc.vector.tensor_copy(out=dbt[:, 40:44], in_=fpc[:])
                nc.vector.tensor_copy(out=dbt[:, 44:48], in_=fpp[:])
                offf = pb.tile([P, RB], F32)
                nc.vector.tensor_copy(out=offf[:], in_=offs[:])
                nc.vector.tensor_copy(out=dbt[:, 48:52], in_=offf[:])
                nc.sync.dma_start(dbg[:, :], dbt[:])

            # ========== phase A (rest) ==================================
            for k in range(SPLIT, NT):
                stream_pair(k)

            # single coalesced raw-partials store, issued from the ACT
            # ring: ACT reaches it in-order right after its own final
            # accumulator-read, skipping the cross-engine semaphore hop
            # a Sync-issued store would pay (the DVE last-slice reduce
            # lands earlier, so its embedded wait is already satisfied).
            nc.scalar.dma_start(partials[:, :], parts1[:])

    return nc


_NC_CACHE = {}


def _get_nc(dx):
    # dx is baked into the program as an immediate (it is on the index
    # chain's critical path; a [128,1] DMA for it completed ~6us late
    # behind the stream's 8KB packets and stalled the pred-window
    # gathers).  Cache is keyed by dx so any grid respacing rebuilds.
    key = float(dx)
    if key not in _NC_CACHE:
        nc = build_nc(key)
        # Bacc runs its compile pipeline (register alloc, sync-wait
        # splitting) in finalize; the PJRT exec path requires it.
        nc.finalize()
        _NC_CACHE[key] = nc
    return _NC_CACHE[key]


def make_in_maps(predicted_solution_batch, target_solution_batch,
                 c_input_batch, x_eval_points):
    pred = np.ascontiguousarray(predicted_solution_batch, dtype=np.float32)
    targ = np.ascontiguousarray(target_solution_batch, dtype=np.float32)
    c = np.ascontiguousarray(c_input_batch, dtype=np.float32)
    x = np.ascontiguousarray(x_eval_points, dtype=np.float32)
    in_maps = []
    for i in range(NCORES):
        sl = slice(i * BL, (i + 1) * BL)
        in_maps.append({
            "pred": pred[sl],
            "targ": targ[sl],
            "cvec": c[sl].reshape(P, RB),
        })
    return in_maps


def reduce_partials(results):
    s = np.zeros(3, dtype=np.float64)
    for r in results:
        p = r["partials"].astype(np.float64)
        s[0] += p[:, :NT].sum()
        s[1] += p[:, NT].sum()
        s[2] += p[:, NT + 1].sum()
    loss = s[0] / (B * N) + s[1] / B + s[2] / B
    return np.float32(loss)


def kernel(predicted_solution_batch, target_solution_batch,
           c_input_batch, x_eval_points):
    x = np.asarray(x_eval_points, dtype=np.float32)
    dx = np.float32(x[1]) - np.float32(x[0])
    nc = _get_nc(dx)
    in_maps = make_in_maps(predicted_solution_batch, target_solution_batch,
                           c_input_batch, x_eval_points)
    res = run_bass_kernel_spmd(nc, in_maps, core_ids=list(range(NCORES)))
    return reduce_partials(res.results)


# revision 44
# speedup vs baseline: 1.0083x; 1.0083x over previous
"""Trainium2 Bass kernel for the CustomODELoss problem.

Full inputs:
    predicted_solution_batch [4096, 8192] f32
    target_solution_batch    [4096, 8192] f32
    c_input_batch            [4096]       f32
    x_eval_points            [8192]       f32   (uniform grid on [0, 1])

loss = mean((pred - target)^2)
     + mean((pred[r, idx_r] - 1)^2)
     + mean(((pred[r, idx_p] - pred[r, idx_m]) / ((idx_p - idx_m) * dx))^2)
where idx_r = argmin_j |x_j - c_r| (first index on ties).

Sharding: data-parallel over the batch dim, 512 rows per core on 8 cores.
Each core streams its pred/targ slice once (memory-bound integral term)
and resolves the per-row grid index + finite-difference gather on device
via indirect DMA.

Key structure (from trace analysis):
  * The HWDGE sync ring sustains ~420 GB/s (16 DMA engines x ~26 GB/s)
    regardless of packet size >= 2KB.  1MB (128x2048) descriptors are the
    sweet spot: 4MB descriptors drop the ring to ~320 GB/s and fire their
    completion semaphores ~2 descriptors late (round-robin expansion);
    small lead-in descriptors engage fewer engines and slow the ramp.
  * Streaming compute: DVE subtract in place, ACT Square with accum_out
    per tile.  The last row block tapers 2048->128 so the post-stream
    serial tail (last load -> sub -> square -> reduce -> store) is short;
    the final tile's square+sum runs on DVE while ACT drains the
    second-to-last accumulates.
  * The index resolve is exact: candidate j0 = int(c*(N-1)) is within 1
    of the true argmin; the three candidate grid values are computed
    ARITHMETICALLY as (jc+e)*dx, which reproduces linspace(0,1,N) f32
    bit-exactly for every j (verified on the full grid), so no x gather
    is needed.  Distance comparison via squares preserves order/ties;
    first-index tie-break matches jnp.argmin.  A 5-wide pred window
    gathered by indirect DMA covers every (idx-1, idx, idx+1) triple.
  * dx is baked into the program as an immediate (a [128,1] DMA for it
    completed ~6us late behind the stream's 8KB packets and stalled the
    index chain); c loads via the sync HWDGE ring ahead of the stream
    (SWDGE would land it ~5us later); pred-window gathers on gpsimd
    SWDGE.  The pw-dependent select chain is anchored mid-stream via a
    bit-exact +0 so a late gather can never stall the in-order DVE queue
    between early stream subtracts.
  * The device emits per-partition partial sums; the host sums the
    8x128 partials and forms the three means.
"""

import numpy as np

import concourse.bacc as bacc
import concourse.bass as bass
import concourse.mybir as mybir
from concourse import tile
from concourse.bass_utils import run_bass_kernel_spmd

F32 = mybir.dt.float32
I32 = mybir.dt.int32
OP = mybir.AluOpType

B = 4096
N = 8192
NCORES = 8
BL = B // NCORES          # rows per core = 512
P = 128                   # SBUF partitions
RB = BL // P              # row groups per partition = 4
FT = 2048                 # max free-dim tile width for the streaming phase
W = 5                     # pred-window width
PBUF = 12                 # stream pool depth (pred/targ each)

# Streaming tile schedule: (row_block, col_start, width) per pred/targ pair.
# The last row block tapers to tiny tiles so the serial pipeline tail
# (last load -> sub -> square -> reduce -> store) is short; everything
# before the taper stays full-width so the stream keeps large transfers.
# (Tried and LOSES: 4MB row-block descriptors — the ring's round-robin
# expansion fires completion semaphores ~2 descriptors late and sustained
# bandwidth drops ~420 -> ~320 GB/s; small ramp-in tiles — the ring
# expands only ~4 descriptors concurrently, so small first descriptors
# engage FEWER engines and slow the ramp.)
_taper = [2048, 2048, 2048, 1024, 512, 256, 128, 64, 64]
assert sum(_taper) == N
TILES = [(_rb, _c * FT, FT) for _rb in range(BL // P - 1) for _c in range(N // FT)]
_cs = 0
for _w in _taper:
    TILES.append((BL // P - 1, _cs, _w))
    _cs += _w
NT = len(TILES)  # 20


def build_nc(dx, debug=False):
    # Bacc (not plain Bass): its compile pipeline runs
    # generate_event_semaphores, which splits multi-sem waits into separate
    # event instructions — TRN2 allows at most 1 embedded wait per
    # instruction, and walrus codegen rejects the unsplit form.
    nc = bacc.Bacc()

    pred = nc.dram_tensor("pred", [BL, N], F32, kind="ExternalInput")
    targ = nc.dram_tensor("targ", [BL, N], F32, kind="ExternalInput")
    # c per core, reshaped host-side to [128, 4]: row r = p*RB + q
    cvec = nc.dram_tensor("cvec", [P, RB], F32, kind="ExternalInput")
    # raw per-slice partials: cols 0..NT-1 = integral-term slice sums,
    # col NT = sum (f(c)-1)^2, col NT+1 = sum f'(c)^2.  The host sums
    # 8x128 partials anyway, so no on-device final reduce is needed —
    # that keeps the post-stream critical path one DVE/ACT op shorter.
    partials = nc.dram_tensor("partials", [P, NT + 2], F32, kind="ExternalOutput")
    if debug:
        dbg = nc.dram_tensor("dbg", [P, 56], F32, kind="ExternalOutput")

    def view3(t):  # [128, 12] tile -> [128, 4, 3] AP
        return t[:].rearrange("p (q k) -> p q k", k=3)

    def view5(t):  # [128, 20] tile -> [128, 4, 5] AP
        return t[:].rearrange("p (q k) -> p q k", k=5)

    with tile.TileContext(nc) as tc:
        with (
            tc.tile_pool(name="ppool", bufs=PBUF) as ppool,
            tc.tile_pool(name="tpool", bufs=PBUF) as tpool,
            tc.tile_pool(name="pb", bufs=1) as pb,
        ):
            # ========== phase A: stream sum((p-t)^2) ====================
            # All stream loads ride the sync HWDGE ring, which carries only
            # pure DMA dispatches and never starves.  (Putting targ loads on
            # the scalar ring was tried and LOSES: the two queues share the
            # same 16 DMA engines, and queue arbitration dips the combined
            # throughput while both are active.)
            parts1 = pb.tile([P, NT + 2], F32)

            tts = []

            def stream_pair(k):
                rb, cs, w = TILES[k]
                rs = rb * P
                pt = ppool.tile([P, FT], F32)
                tt = tpool.tile([P, FT], F32)
                tts.append(tt)
                nc.sync.dma_start(pt[:, :w], pred[rs:rs + P, cs:cs + w])
                if k == 0:
                    # tile 0's targ load warms the ACT HWDGE ring (neutral
                    # for the stream — verified) so the final store below
                    # doesn't pay the ring's cold-start latency.
                    nc.scalar.dma_start(tt[:, :w], targ[rs:rs + P, cs:cs + w])
                else:
                    nc.sync.dma_start(tt[:, :w], targ[rs:rs + P, cs:cs + w])
                # in-place pt <- pt - tt (the freed third pool doubles the
                # stream-pool depth instead)
                nc.vector.tensor_tensor(out=pt[:, :w], in0=pt[:, :w],
                                        in1=tt[:, :w], op=OP.subtract)
                if k >= NT - 3:
                    # Last three (tiny) tiles square+sum on DVE: the serial
                    # tail after the final DMA byte is then a short DVE
                    # chain running in parallel with ACT draining the
                    # bigger taper squares, instead of ACT serializing
                    # three more sq+accum-read pairs (~0.3us each read)
                    # behind them.
                    nc.vector.tensor_tensor(out=pt[:, :w], in0=pt[:, :w],
                                            in1=pt[:, :w], op=OP.mult)
                    nc.vector.reduce_sum(out=parts1[:, k:k + 1],
                                         in_=pt[:, :w],
                                         axis=mybir.AxisListType.X)
                else:
                    # pt <- pt^2 in place; accum_out = row-sum
                    nc.scalar.activation(
                        out=pt[:, :w], in_=pt[:, :w],
                        func=mybir.ActivationFunctionType.Square,
                        accum_out=parts1[:, k:k + 1],
                    )

            stream_pair(0)

            # ========== phase B part 1: indices + pred-window gather =====
            # c rides the sync HWDGE ring right AFTER pred0's dispatch:
            # via SWDGE it lands only ~13.5us in, which holds up the index
            # chain -> pw gathers -> the pw-gated part-2 DVE ops — via the
            # sync ring c lands ~9.5us and the resolve finishes with slack.
            # pred0 dispatches first so the stream is not delayed.
            c_t = pb.tile([P, RB], F32)
            nc.sync.dma_start(out=c_t[:], in_=cvec[:, :])

            # j0 = int(c * (N-1)); any convert rounding mode keeps
            # |j0 - argmin| <= 1, which the 3-candidate check fixes.
            u = pb.tile([P, RB], F32)
            nc.vector.tensor_scalar(out=u[:], in0=c_t[:], scalar1=float(N - 1),
                                    scalar2=None, op0=OP.mult)
            j0i = pb.tile([P, RB], I32)
            nc.vector.tensor_copy(out=j0i[:], in_=u[:])
            j0f = pb.tile([P, RB], F32)
            nc.vector.tensor_copy(out=j0f[:], in_=j0i[:])
            jcc = pb.tile([P, RB], F32)
            nc.vector.tensor_scalar(out=jcc[:], in0=j0f[:], scalar1=1.0,
                                    scalar2=float(N - 2), op0=OP.max, op1=OP.min)

            # pred window start: clip(j0-2, 0, N-W) — the 5-wide window
            # covers {jm, jstar, jp} for every jstar in {j0-1, j0, j0+1}.
            s5f = pb.tile([P, RB], F32)
            nc.vector.tensor_scalar(out=s5f[:], in0=j0f[:], scalar1=-2.0,
                                    scalar2=0.0, op0=OP.add, op1=OP.max)
            s5c = pb.tile([P, RB], F32)
            nc.vector.tensor_scalar(out=s5c[:], in0=s5f[:],
                                    scalar1=float(N - W), scalar2=None,
                                    op0=OP.min)
            s5i = pb.tile([P, RB], I32)
            nc.vector.tensor_copy(out=s5i[:], in_=s5c[:])
            rowbase = pb.tile([P, RB], I32)  # (p*RB + q) * N
            nc.gpsimd.iota(rowbase[:], pattern=[[N, RB]], base=0,
                           channel_multiplier=RB * N)
            offs = pb.tile([P, RB], I32)
            nc.vector.tensor_tensor(out=offs[:], in0=rowbase[:], in1=s5i[:],
                                    op=OP.add)

            # NOTE: hardware SWDGE honors only ONE offset per partition in an
            # indirect DMA (CoreSim accepts [128, RB] offsets, HW does not) —
            # issue one gather per row-group with [128, 1] offsets.
            pw = pb.tile([P, RB * W], F32)
            for q in range(RB):
                nc.gpsimd.indirect_dma_start(
                    out=pw[:, W * q:W * q + W], out_offset=None,
                    in_=pred[:, :],
                    in_offset=bass.IndirectOffsetOnAxis(
                        ap=offs[:, q:q + 1], axis=1),
                )

            iota15 = pb.tile([P, RB * W], F32)
            nc.gpsimd.iota(iota15[:], pattern=[[0, RB], [1, W]], base=0,
                           channel_multiplier=0,
                           allow_small_or_imprecise_dtypes=True)

            SPLIT = 12
            for k in range(1, SPLIT):
                stream_pair(k)

            # Anchor for the pw-dependent ops below: a zero tile whose input
            # is stream pair 8's targ buffer (read-only — no WAR on the
            # in-place subtract).  pw2 = pw + 0 is bit-identical to pw, but
            # it forces the scheduler to place the one-hot select chain
            # mid-stream (~45us), when the SWDGE gathers are guaranteed
            # complete even on a bad day.  Without it the in-order DVE queue
            # holds pw-gated ops right after the first stream subtract, and
            # a late gather stalls DVE -> the op-count dispatch guards fire
            # late -> the ring starves -> completion lateness compounds
            # (the bimodal ~97 vs ~114 us runs).
            z20 = pb.tile([P, RB * W], F32)
            nc.vector.tensor_scalar(out=z20[:], in0=tts[8][:, :RB * W],
                                    scalar1=0.0, scalar2=None, op0=OP.mult)
            pw2 = pb.tile([P, RB * W], F32)
            nc.vector.tensor_tensor(out=pw2[:], in0=pw[:], in1=z20[:],
                                    op=OP.add)

            # ========== phase B part 2: select + finite difference ======
            # Candidate grid values computed arithmetically: x_j = fl(j*dx)
            # bit-exactly reproduces linspace(0,1,N) f32 for every j
            # (verified on the full grid), so no x gather is needed — the
            # three candidates {jc-1, jc, jc+1} are (jcc+e)*dx.
            xw = pb.tile([P, RB * 3], F32)
            for e in range(3):
                nc.vector.tensor_scalar(out=xw[:, e::3], in0=jcc[:],
                                        scalar1=float(e - 1),
                                        scalar2=dx,
                                        op0=OP.add, op1=OP.mult)
            # Pure DVE (no ACT hop): |d| comparisons use d*d — f32 squaring
            # is monotone in |d|, so order and ties match abs comparison.
            dsb = pb.tile([P, RB * 3], F32)
            nc.vector.tensor_tensor(out=view3(dsb), in0=view3(xw),
                                    in1=c_t[:].to_broadcast([P, RB, 3]),
                                    op=OP.subtract)
            dsq = pb.tile([P, RB * 3], F32)
            nc.vector.tensor_tensor(out=dsq[:], in0=dsb[:], in1=dsb[:],
                                    op=OP.mult)
            dm, d0, dp = dsq[:, 0::3], dsq[:, 1::3], dsq[:, 2::3]

            # first-argmin among {jc-1, jc, jc+1}:
            #   a = (dm<=d0)&(dm<=dp); b = (1-a)&(d0<=dp)
            #   jstar = jc + 1 - 2a - b
            t1b = pb.tile([P, RB], F32)
            nc.vector.tensor_tensor(out=t1b[:], in0=dm, in1=d0, op=OP.is_le)
            t2b = pb.tile([P, RB], F32)
            nc.vector.tensor_tensor(out=t2b[:], in0=dm, in1=dp, op=OP.is_le)
            a_t = pb.tile([P, RB], F32)
            nc.vector.tensor_tensor(out=a_t[:], in0=t1b[:], in1=t2b[:],
                                    op=OP.mult)
            t3b = pb.tile([P, RB], F32)
            nc.vector.tensor_tensor(out=t3b[:], in0=d0, in1=dp, op=OP.is_le)
            oma = pb.tile([P, RB], F32)
            nc.vector.tensor_scalar(out=oma[:], in0=a_t[:], scalar1=-1.0,
                                    scalar2=1.0, op0=OP.mult, op1=OP.add)
            b_t = pb.tile([P, RB], F32)
            nc.vector.tensor_tensor(out=b_t[:], in0=t3b[:], in1=oma[:],
                                    op=OP.mult)
            e1 = pb.tile([P, RB], F32)
            nc.vector.tensor_scalar(out=e1[:], in0=a_t[:], scalar1=-2.0,
                                    scalar2=1.0, op0=OP.mult, op1=OP.add)
            e2 = pb.tile([P, RB], F32)
            nc.vector.tensor_tensor(out=e2[:], in0=e1[:], in1=b_t[:],
                                    op=OP.subtract)
            jstar = pb.tile([P, RB], F32)
            nc.vector.tensor_tensor(out=jstar[:], in0=jcc[:], in1=e2[:],
                                    op=OP.add)

            # neighbors and in-window positions relative to s5
            jm = pb.tile([P, RB], F32)
            nc.vector.tensor_scalar(out=jm[:], in0=jstar[:], scalar1=-1.0,
                                    scalar2=0.0, op0=OP.add, op1=OP.max)
            jp = pb.tile([P, RB], F32)
            nc.vector.tensor_scalar(out=jp[:], in0=jstar[:], scalar1=1.0,
                                    scalar2=float(N - 1), op0=OP.add, op1=OP.min)
            p0 = pb.tile([P, RB], F32)
            nc.vector.tensor_tensor(out=p0[:], in0=jstar[:], in1=s5c[:],
                                    op=OP.subtract)
            pmp = pb.tile([P, RB], F32)
            nc.vector.tensor_tensor(out=pmp[:], in0=jm[:], in1=s5c[:],
                                    op=OP.subtract)
            ppp = pb.tile([P, RB], F32)
            nc.vector.tensor_tensor(out=ppp[:], in0=jp[:], in1=s5c[:],
                                    op=OP.subtract)

            # f(c): one-hot select of window position jstar
            m0 = pb.tile([P, RB * W], F32)
            nc.vector.tensor_tensor(out=view5(m0), in0=view5(iota15),
                                    in1=p0[:].to_broadcast([P, RB, W]),
                                    op=OP.is_equal)
            pr0 = pb.tile([P, RB * W], F32)
            nc.vector.tensor_tensor(out=pr0[:], in0=m0[:], in1=pw2[:],
                                    op=OP.mult)
            fpc = pb.tile([P, RB], F32)
            nc.vector.reduce_sum(out=fpc[:], in_=view5(pr0),
                                 axis=mybir.AxisListType.X)

            # f'(c): (pred[jp] - pred[jm]) / ((jp-jm)*dx) via +/- one-hot
            mp_ = pb.tile([P, RB * W], F32)
            nc.vector.tensor_tensor(out=view5(mp_), in0=view5(iota15),
                                    in1=ppp[:].to_broadcast([P, RB, W]),
                                    op=OP.is_equal)
            mm_ = pb.tile([P, RB * W], F32)
            nc.vector.tensor_tensor(out=view5(mm_), in0=view5(iota15),
                                    in1=pmp[:].to_broadcast([P, RB, W]),
                                    op=OP.is_equal)
            wd = pb.tile([P, RB * W], F32)
            nc.vector.tensor_tensor(out=wd[:], in0=mp_[:], in1=mm_[:],
                                    op=OP.subtract)
            prd = pb.tile([P, RB * W], F32)
            nc.vector.tensor_tensor(out=prd[:], in0=wd[:], in1=pw2[:],
                                    op=OP.mult)
            df = pb.tile([P, RB], F32)
            nc.vector.reduce_sum(out=df[:], in_=view5(prd),
                                 axis=mybir.AxisListType.X)
            qd = pb.tile([P, RB], F32)
            nc.vector.tensor_tensor(out=qd[:], in0=jp[:], in1=jm[:],
                                    op=OP.subtract)
            den = pb.tile([P, RB], F32)
            nc.vector.tensor_scalar(out=den[:], in0=qd[:], scalar1=dx,
                                    scalar2=None, op0=OP.mult)
            rden = pb.tile([P, RB], F32)
            nc.vector.reciprocal(out=rden[:], in_=den[:])
            fpp = pb.tile([P, RB], F32)
            nc.vector.tensor_tensor(out=fpp[:], in0=df[:], in1=rden[:],
                                    op=OP.mult)

            # per-partition sums of (f(c)-1)^2 and f'(c)^2, straight into
            # the coalesced output tile.  (tensor_tensor_reduce compiles
            # but dies at runtime on HW — use ACT Square with accumulate
            # instead; these are terminal outputs, so the ACT-stream
            # position doesn't gate anything.)
            fpm1 = pb.tile([P, RB], F32)
            nc.vector.tensor_scalar(out=fpm1[:], in0=fpc[:], scalar1=-1.0,
                                    scalar2=None, op0=OP.add)
            sq2 = pb.tile([P, RB], F32)
            nc.scalar.activation(out=sq2[:], in_=fpm1[:],
                                 func=mybir.ActivationFunctionType.Square,
                                 accum_out=parts1[:, NT:NT + 1])
            sq3 = pb.tile([P, RB], F32)
            nc.scalar.activation(out=sq3[:], in_=fpp[:],
                                 func=mybir.ActivationFunctionType.Square,
                                 accum_out=parts1[:, NT + 1:NT + 2])

            if debug:
                dbt = pb.tile([P, 56], F32)
                nc.vector.tensor_copy(out=dbt[:, 0:12], in_=xw[:])
                nc.vector.tensor_copy(out=dbt[:, 12:32], in_=pw[:])
                nc.vector.tensor_copy(out=dbt[:, 32:36], in_=jstar[:])
                nc.vector.tensor_copy(out=dbt[:, 36:40], in_=s5c[:])
                nc.vector.tensor_copy(out=dbt[:, 40:44], in_=fpc[:])
                nc.vector.tensor_copy(out=dbt[:, 44:48], in_=fpp[:])
                offf = pb.tile([P, RB], F32)
                nc.vector.tensor_copy(out=offf[:], in_=offs[:])
                nc.vector.tensor_copy(out=dbt[:, 48:52], in_=offf[:])
                nc.sync.dma_start(dbg[:, :], dbt[:])

            # ========== phase A (rest) ==================================
            for k in range(SPLIT, NT):
                stream_pair(k)

            # single coalesced raw-partials store, issued from the ACT
            # ring: ACT reaches it in-order right after its own final
            # accumulator-read, skipping the cross-engine semaphore hop
            # a Sync-issued store would pay (the DVE last-slice reduce
            # lands earlier, so its embedded wait is already satisfied).
            nc.scalar.dma_start(partials[:, :], parts1[:])

    return nc


_NC_CACHE = {}


def _get_nc(dx):
    # dx is baked into the program as an immediate (it is on the index
    # chain's critical path; a [128,1] DMA for it completed ~6us late
    # behind the stream's 8KB packets and stalled the pred-window
    # gathers).  Cache is keyed by dx so any grid respacing rebuilds.
    key = float(dx)
    if key not in _NC_CACHE:
        nc = build_nc(key)
        # Bacc runs its compile pipeline (register alloc, sync-wait
        # splitting) in finalize; the PJRT exec path requires it.
        nc.finalize()
        _NC_CACHE[key] = nc
    return _NC_CACHE[key]


def make_in_maps(predicted_solution_batch, target_solution_batch,
                 c_input_batch, x_eval_points):
    pred = np.ascontiguousarray(predicted_solution_batch, dtype=np.float32)
    targ = np.ascontiguousarray(target_solution_batch, dtype=np.float32)
    c = np.ascontiguousarray(c_input_batch, dtype=np.float32)
    x = np.ascontiguousarray(x_eval_points, dtype=np.float32)
    in_maps = []
    for i in range(NCORES):
        sl = slice(i * BL, (i + 1) * BL)
        in_maps.append({
            "pred": pred[sl],
            "targ": targ[sl],
            "cvec": c[sl].reshape(P, RB),
        })
    return in_maps


def reduce_partials(results):
    s = np.zeros(3, dtype=np.float64)
    for r in results:
        p = r["partials"].astype(np.float64)
        s[0] += p[:, :NT].sum()
        s[1] += p[:, NT].sum()
        s[2] += p[:, NT + 1].sum()
    loss = s[0] / (B * N) + s[1] / B + s[2] / B
    return np.float32(loss)


def kernel(predicted_solution_batch, target_solution_batch,
           c_input_batch, x_eval_points):
    x = np.asarray(x_eval_points, dtype=np.float32)
    dx = np.float32(x[1]) - np.float32(x[0])
    nc = _get_nc(dx)
    in_maps = make_in_maps(predicted_solution_batch, target_solution_batch,
                           c_input_batch, x_eval_points)
    res = run_bass_kernel_spmd(nc, in_maps, core_ids=list(range(NCORES)))
    return reduce_partials(res.results)
